# revision 1
# baseline (speedup 1.0000x reference)
"""Trainium2 Bass kernel for AttentionalPositionEncoding.

Reference computation (per batch b, with x_tok = x.reshape(C, N).T):
    cnn   = x_tok @ Wc.T
    q     = cnn @ Wq.T + bq           -> heads [h=8, N=1024, dk=32]
    k     = pos @ Wk.T + bk
    v     = pos @ Wv.T + bv
    attn  = softmax(q k^T / sqrt(dk)) @ v
    out   = (cnn @ Wf.T + bf + attn) @ Wo.T + bo + x_tok

Sharding: data-parallel over B=8 across the 8 NeuronCores (1 batch/core).

Host-side weight folding (exact algebra, done in fp32):
    Wqc  = Wq @ Wc          (q     = x_tok @ Wqc.T + bq)
    Wofc = Wo @ Wf @ Wc     (ffn   = x_tok @ Wofc.T)
    bfo  = Wo @ bf + bo

On-chip layout is feature-major ("CN": features on partitions, tokens on
free dim), which matches the HBM layout of x/pos ([C, H*W]) so no input
transposes are needed.  Attention scores are computed *transposed*
(S^T[j, i], keys on psum partitions) so that exp(S^T) feeds the P@V
matmul directly as the moving operand.  Softmax is unnormalized
(scores are O(8), exp is safe in fp32); the row sums Z are produced by a
ones-column appended to V (M=33 stationary), and 1/Z is applied after
P@V via a small select-matrix broadcast matmul.

The default execution path is the "batched" variant: per head pair, all
32 score matmuls + 16 [128,1024] exps run first (single PE tile mode,
ACT-paced), then all 32 P@V matmuls run back-to-back with contiguous
PSUM accumulation chains.  All matmuls are fp32r (full-rate fp32),
measured end-to-end relative error vs the fp32 reference: 1.6e-4.
Local measurement: ~190 us/kernel on one NeuronCore (8 cores run
data-parallel, one batch each).
"""

import math

import numpy as np

import concourse.bacc as bacc
import concourse.mybir as mybir
import concourse.tile as tile
from concourse.bass_utils import run_bass_kernel_spmd

F32 = mybir.dt.float32
F32R = mybir.dt.float32r
BF16 = mybir.dt.bfloat16

D = 256          # d_model
H = 8            # heads
DK = 32          # head dim
N = 1024         # tokens (32*32)
NCORES = 8
SCALE = 1.0 / math.sqrt(DK)


def _r(ap):
    """Bitcast an AP to float32r so the PE runs at 1 cycle/row."""
    return ap.bitcast(F32R)


def build(loop_input=False, variant="full"):
    """Build the per-core Bass program.

    loop_input=True adds a uint32 [1,1] input "niter" and wraps the whole
    body in a dynamic For_i — used by the local timing harness only.
    variant: "full" | ablations used for local perf attribution.
    """
    nc = bacc.Bacc(None, target_bir_lowering=False)

    x_d = nc.dram_tensor("x", [D, N], F32R, kind="ExternalInput")
    pos_d = nc.dram_tensor("pos", [D, N], F32R, kind="ExternalInput")
    wqcT_d = nc.dram_tensor("wqcT", [D, D], F32R, kind="ExternalInput")
    wkT_d = nc.dram_tensor("wkT", [D, D], F32R, kind="ExternalInput")
    # v weights augmented with a zero 33rd column per head; the ones come
    # from the bias row, so P@V also produces the softmax denominators Z.
    wvT_d = nc.dram_tensor("wvT", [D, H * (DK + 1)], F32R, kind="ExternalInput")
    wofcT_d = nc.dram_tensor("wofcT", [D, D], F32R, kind="ExternalInput")
    woT_d = nc.dram_tensor("woT", [D, D], F32R, kind="ExternalInput")
    bpp_d = nc.dram_tensor("b_pp", [128, 6], F32, kind="ExternalInput")
    brow_d = nc.dram_tensor("b_row", [1, H * (DK + 1)], F32R,
                            kind="ExternalInput")
    ones_d = nc.dram_tensor("ones1", [1, 128], F32R, kind="ExternalInput")
    out_d = nc.dram_tensor("out", [D, N], F32, kind="ExternalOutput")
    if loop_input:
        niter_d = nc.dram_tensor("niter", [1, 1], mybir.dt.uint32,
                                 kind="ExternalInput")

    with tile.TileContext(nc) as tc:
        import contextlib
        with contextlib.ExitStack() as stk:
            if loop_input:
                cpool = stk.enter_context(tc.tile_pool(name="cfg", bufs=1))
                nit_sb = cpool.tile([1, 1], mybir.dt.uint32)
                nc.sync.dma_start(nit_sb[:], niter_d[:])
                nit = nc.values_load(nit_sb[0:1, 0:1], min_val=1,
                                     max_val=1 << 20,
                                     skip_runtime_bounds_check=True)
                loop_cm = tc.For_i(0, nit, 1,
                                   hint_engines=tuple(mybir.ALL_ENGINES))
            else:
                loop_cm = contextlib.nullcontext()
            with loop_cm:
                _body(nc, tc, x_d, pos_d, wqcT_d, wkT_d, wvT_d, wofcT_d,
                      woT_d, bpp_d, brow_d, ones_d, out_d, variant)
    nc.compile()
    return nc


def _body(nc, tc, x_d, pos_d, wqcT_d, wkT_d, wvT_d, wofcT_d, woT_d,
          bpp_d, brow_d, ones_d, out_d, variant="full"):
    import contextlib
    with contextlib.ExitStack() as stk:
        ep = stk.enter_context

        persist = ep(tc.tile_pool(name="persist", bufs=1))

        # ---------- load inputs ----------
        def load_cn(dram, name):
            t = persist.tile([128, 2, dram.shape[1]], F32R, tag=name)
            nc.sync.dma_start(t[:], dram[:].rearrange("(k p) n -> p k n", p=128))
            return t

        x_sb = load_cn(x_d, "x_sb")          # [128, kt, 1024]
        pos_sb = load_cn(pos_d, "pos_sb")
        wqc_sb = load_cn(wqcT_d, "wqc_sb")   # [128, kt, 256]
        wk_sb = load_cn(wkT_d, "wk_sb")
        wv_sb = load_cn(wvT_d, "wv_sb")
        wofc_sb = load_cn(wofcT_d, "wofc_sb")
        wo_sb = load_cn(woT_d, "wo_sb")
        bpp = persist.tile([128, 6], F32, tag="bpp")
        nc.sync.dma_start(bpp[:], bpp_d[:])
        brow = persist.tile([1, H * (DK + 1)], F32R, tag="brow")
        nc.sync.dma_start(brow[:], brow_d[:])

        # constants
        ones1 = persist.tile([1, 128], F32R, tag="ones1")
        nc.sync.dma_start(ones1[:], ones_d[:])
        zbias = persist.tile([128, 1], F32, tag="zbias")
        nc.gpsimd.memset(zbias[:], 0.0)

        # persistent activations
        v2 = variant.startswith("v2") or variant == "pipelined"
        pv_dt = BF16 if ("bf16pv" in variant or v2) else F32R
        qk_dt = BF16 if variant.startswith("v2") else F32R
        q_sb = persist.tile([128, 2, N], qk_dt, tag="q_sb")
        k_sb = persist.tile([128, 2, N], qk_dt, tag="k_sb")
        v_aug = persist.tile([128, 8, H * (DK + 1)], pv_dt, tag="v_aug")
        oT_sb = persist.tile([128, 2, N], F32R, tag="oT_sb")
        # 1/Z rows, all on partition 0 (engine APs need 32-aligned bases)
        zinv = persist.tile([1, H, N], F32R, tag="zinv")
        out_sb = persist.tile([128, 2, N], F32, tag="out_sb")

        # ---------- q / k projections (CN layout) ----------
        with tc.tile_pool(name="dense_ps", bufs=2, space="PSUM") as dense_ps:
            for (dst, w_sb, rhs_sb, bcol) in ((q_sb, wqc_sb, x_sb, 0),
                                              (k_sb, wk_sb, pos_sb, 2)):
                for mt in range(2):
                    for ch in range(2):
                        ps = dense_ps.tile([128, 512], F32, tag="dense")
                        for kt in range(2):
                            nc.tensor.matmul(
                                ps[:],
                                _r(w_sb[:, kt, 128 * mt:128 * mt + 128]),
                                _r(rhs_sb[:, kt, 512 * ch:512 * ch + 512]),
                                start=(kt == 0), stop=(kt == 1))
                        with nc.allow_low_precision(reason="qk dtype knob"):
                            nc.vector.tensor_scalar_add(
                                dst[:, mt, 512 * ch:512 * ch + 512], ps[:],
                                bpp[:, bcol + mt:bcol + mt + 1])

            # ------- v projection (token-major, ones-augmented) -------
            for jt in range(8):
                ps = dense_ps.tile([128, H * (DK + 1)], F32, tag="dense")
                for kt in range(2):
                    nc.tensor.matmul(
                        ps[:],
                        _r(pos_sb[:, kt, 128 * jt:128 * jt + 128]),
                        _r(wv_sb[:, kt, :]),
                        start=(kt == 0), stop=False)
                nc.tensor.matmul(ps[:], _r(ones1[:]), _r(brow[:]),
                                 start=False, stop=True)
                with nc.allow_low_precision(reason="pv dtype knob"):
                    nc.vector.tensor_copy(v_aug[:, jt, :], ps[:])

        # ---------- attention: head pairs ----------
        attn_stk = stk.enter_context(contextlib.ExitStack())
        sc_ps = attn_stk.enter_context(
            tc.tile_pool(name="sc_ps", bufs=2, space="PSUM"))
        pv_ps = attn_stk.enter_context(
            tc.tile_pool(name="pv_ps", bufs=1, space="PSUM"))
        n_et = 34 if variant == "pipelined" else (
            18 if (variant.startswith("batched")
                   or variant.startswith("v2")) else 3)
        e_pool = attn_stk.enter_context(tc.tile_pool(name="e_pool", bufs=n_et))

        if variant.startswith("v2") and variant != "v2noattn":
            # bf16 attention: per-head score tiles with N=1024 streams.
            for hp in range(4):
                hA, hB = 2 * hp, 2 * hp + 1
                dt = hA // 4
                pA, pB = 32 * (hA % 4), 32 * (hB % 4)
                pvA = pv_ps.tile([128, N], F32, tag="pvA")
                pvB = pv_ps.tile([128, N], F32, tag="pvB")
                ets = {}
                # phase 1: scores + exp, one PE mode (32-row tiles)
                for jt in range(8):
                    for (h0, p0) in ((hA, pA), (hB, pB)):
                        sc = sc_ps.tile([128, 1024], F32, tag="sc")
                        nc.tensor.matmul(
                            sc[:],
                            k_sb[p0:p0 + 32, dt, 128 * jt:128 * jt + 128],
                            q_sb[p0:p0 + 32, dt, :],
                            start=True, stop=True, tile_position=(p0, 0))
                        et = e_pool.tile([128, 1024], BF16, tag="et")
                        with nc.allow_low_precision(reason="bf16 attention"):
                            nc.scalar.activation(
                                et[:], sc[:],
                                mybir.ActivationFunctionType.Exp,
                                bias=zbias[:, 0:1], scale=SCALE)
                        ets[(h0, jt)] = et
                # phase 2: P@V, one PE mode, contiguous accum chains
                for (h0, pvt) in ((hA, pvA), (hB, pvB)):
                    for jt in range(8):
                        nc.tensor.matmul(
                            pvt[0:DK + 1, :],
                            v_aug[:, jt,
                                  (DK + 1) * h0:(DK + 1) * h0 + DK + 1],
                            ets[(h0, jt)][:],
                            start=(jt == 0), stop=(jt == 7),
                            tile_position=(0, 0))
                nc.vector.tensor_copy(oT_sb[pA:pA + 32, dt, :], pvA[0:32, :])
                nc.vector.tensor_copy(oT_sb[pB:pB + 32, dt, :], pvB[0:32, :])
                with nc.allow_low_precision(reason="f32r full 32-bit width"):
                    nc.vector.reciprocal(zinv[0:1, hA, :], pvA[32:33, :])
                    nc.vector.reciprocal(zinv[0:1, hB, :], pvB[32:33, :])

        if variant == "pipelined":
            # Software-pipelined pairs: pair p's P@V matmuls are emitted
            # after pair p+1's scores+exp, so the PE fills the ACT-paced
            # stretches with P@V work instead of idling.  E and V in bf16
            # so two pairs of exp outputs fit in SBUF.
            def phase1(hp):
                hA, hB = 2 * hp, 2 * hp + 1
                dt = hA // 4
                pA, pB = 32 * (hA % 4), 32 * (hB % 4)
                ets = []
                for jt in range(8):
                    for ch in range(2):
                        sc = sc_ps.tile([128, 1024], F32, tag="sc")
                        for (h0, p0, lo) in ((hA, pA, 0), (hB, pB, 512)):
                            nc.tensor.matmul(
                                sc[:, lo:lo + 512],
                                _r(k_sb[p0:p0 + 32, dt,
                                        128 * jt:128 * jt + 128]),
                                _r(q_sb[p0:p0 + 32, dt,
                                        512 * ch:512 * ch + 512]),
                                start=True, stop=True, tile_position=(p0, 0))
                        et = e_pool.tile([128, 1024], BF16, tag="et")
                        with nc.allow_low_precision(reason="bf16 E"):
                            nc.scalar.activation(
                                et[:], sc[:],
                                mybir.ActivationFunctionType.Exp,
                                bias=zbias[:, 0:1], scale=SCALE)
                        ets.append(et)
                return ets

            def phase2(hp, ets):
                hA, hB = 2 * hp, 2 * hp + 1
                dt = hA // 4
                pA, pB = 32 * (hA % 4), 32 * (hB % 4)
                pvA = pv_ps.tile([128, N], F32, tag="pvA")
                pvB = pv_ps.tile([128, N], F32, tag="pvB")
                for (h0, elo, pvt) in ((hA, 0, pvA), (hB, 512, pvB)):
                    for ch in range(2):
                        for jt in range(8):
                            nc.tensor.matmul(
                                pvt[0:DK + 1, 512 * ch:512 * ch + 512],
                                v_aug[:, jt,
                                      (DK + 1) * h0:(DK + 1) * h0 + DK + 1],
                                ets[jt * 2 + ch][:, elo:elo + 512],
                                start=(jt == 0), stop=(jt == 7),
                                tile_position=(0, 0))
                nc.vector.tensor_copy(oT_sb[pA:pA + 32, dt, :], pvA[0:32, :])
                nc.vector.tensor_copy(oT_sb[pB:pB + 32, dt, :], pvB[0:32, :])
                with nc.allow_low_precision(reason="f32r full width"):
                    nc.vector.reciprocal(zinv[0:1, hA, :], pvA[32:33, :])
                    nc.vector.reciprocal(zinv[0:1, hB, :], pvB[32:33, :])

            prev = None
            for hp in range(4):
                ets = phase1(hp)
                if prev is not None:
                    phase2(prev[0], prev[1])
                prev = (hp, ets)
            phase2(prev[0], prev[1])

        if variant.startswith("batched"):
            for hp in range(4):
                hA, hB = 2 * hp, 2 * hp + 1
                dt = hA // 4
                pA, pB = 32 * (hA % 4), 32 * (hB % 4)
                pvA = pv_ps.tile([128, N], F32, tag="pvA")
                pvB = pv_ps.tile([128, N], F32, tag="pvB")
                ets = []
                # phase 1: all scores + exp for the pair (one PE mode)
                for jt in range(8):
                    for ch in range(2):
                        sc = sc_ps.tile([128, 1024], F32, tag="sc")
                        for (h0, p0, lo) in ((hA, pA, 0), (hB, pB, 512)):
                            nc.tensor.matmul(
                                sc[:, lo:lo + 512],
                                _r(k_sb[p0:p0 + 32, dt,
                                        128 * jt:128 * jt + 128]),
                                _r(q_sb[p0:p0 + 32, dt,
                                        512 * ch:512 * ch + 512]),
                                start=True, stop=True, tile_position=(p0, 0))
                        et = e_pool.tile([128, 1024], pv_dt, tag="et")
                        with nc.allow_low_precision(reason="pv dtype knob"):
                            nc.scalar.activation(
                                et[:], sc[:],
                                mybir.ActivationFunctionType.Exp,
                                bias=zbias[:, 0:1], scale=SCALE)
                        ets.append(et)
                # phase 2: all P@V for the pair (one PE mode, each psum
                # region's accumulation chain contiguous: BEGIN..MID..END)
                for (h0, elo, pvt) in ((hA, 0, pvA), (hB, 512, pvB)):
                    for ch in range(2):
                        for jt in range(8):
                            et = ets[jt * 2 + ch]
                            grp = ((jt == 0), (jt == 7))
                            if variant == "batched_nogroup":
                                grp = (True, True)
                            nc.tensor.matmul(
                                pvt[0:DK + 1, 512 * ch:512 * ch + 512],
                                v_aug[:, jt,
                                      (DK + 1) * h0:(DK + 1) * h0 + DK + 1],
                                et[:, elo:elo + 512],
                                start=grp[0], stop=grp[1],
                                tile_position=(0, 0))
                nc.vector.tensor_copy(oT_sb[pA:pA + 32, dt, :], pvA[0:32, :])
                nc.vector.tensor_copy(oT_sb[pB:pB + 32, dt, :], pvB[0:32, :])
                with nc.allow_low_precision(reason="f32r full 32-bit width"):
                    nc.vector.reciprocal(zinv[0:1, hA, :], pvA[32:33, :])
                    nc.vector.reciprocal(zinv[0:1, hB, :], pvB[32:33, :])

        for hp in (() if (variant.startswith("batched") or v2) else range(4)):
            hA, hB = 2 * hp, 2 * hp + 1
            dt = hA // 4
            pA, pB = 32 * (hA % 4), 32 * (hB % 4)
            # fp32r matmul dst base partition must be 0 (or 64 with M<=32):
            # give each head its own psum tile, both written at (0, 0), M=33.
            pvA = pv_ps.tile([128, N], F32, tag="pvA")
            pvB = pv_ps.tile([128, N], F32, tag="pvB")
            for jt in range(8):
                for ch in range(2):
                    sc = sc_ps.tile([128, 1024], F32, tag="sc")
                    # S^T tiles for heads A (cols 0:512) and B (cols 512:1024)
                    for (h0, p0, lo) in ((hA, pA, 0), (hB, pB, 512)):
                        nc.tensor.matmul(
                            sc[:, lo:lo + 512],
                            _r(k_sb[p0:p0 + 32, dt, 128 * jt:128 * jt + 128]),
                            _r(q_sb[p0:p0 + 32, dt, 512 * ch:512 * ch + 512]),
                            start=True, stop=True, tile_position=(p0, 0))
                    et = e_pool.tile([128, 1024], F32R, tag="et")
                    if variant == "expdve":
                        with nc.allow_low_precision(reason="perf ablation"):
                            nc.vector.tensor_copy(et[:], sc[:])
                    elif variant == "exp_sbuf":
                        st = e_pool.tile([128, 1024], F32, tag="st")
                        with nc.allow_low_precision(reason="perf ablation"):
                            nc.vector.tensor_copy(st[:], sc[:])
                        nc.scalar.activation(et[:], st[:],
                                             mybir.ActivationFunctionType.Exp,
                                             bias=zbias[:, 0:1], scale=SCALE)
                    else:
                        nc.scalar.activation(et[:], sc[:],
                                             mybir.ActivationFunctionType.Exp,
                                             bias=zbias[:, 0:1], scale=SCALE)
                    # P@V (+Z row at 32): accumulate over jt
                    if variant == "sconly":
                        if jt == 0:
                            for pvt in (pvA, pvB):
                                nc.tensor.matmul(
                                    pvt[0:DK + 1, 512 * ch:512 * ch + 512],
                                    _r(v_aug[:, jt, 0:DK + 1]),
                                    _r(et[:, 0:512]),
                                    start=True, stop=True,
                                    tile_position=(0, 0))
                    else:
                        for (h0, elo, pvt) in ((hA, 0, pvA), (hB, 512, pvB)):
                            grp = ((jt == 0), (jt == 7))
                            if variant == "batched_nogroup":
                                grp = (True, True)
                            nc.tensor.matmul(
                                pvt[0:DK + 1, 512 * ch:512 * ch + 512],
                                v_aug[:, jt,
                                      (DK + 1) * h0:(DK + 1) * h0 + DK + 1],
                                et[:, elo:elo + 512],
                                start=grp[0], stop=grp[1],
                                tile_position=(0, 0))
            # evacuate: unnormalized attn^T (CN) + 1/Z rows
            nc.vector.tensor_copy(oT_sb[pA:pA + 32, dt, :], pvA[0:32, :])
            nc.vector.tensor_copy(oT_sb[pB:pB + 32, dt, :], pvB[0:32, :])
            with nc.allow_low_precision(reason="f32r is full 32-bit width"):
                nc.vector.reciprocal(zinv[0:1, hA, :], pvA[32:33, :])
                nc.vector.reciprocal(zinv[0:1, hB, :], pvB[32:33, :])

        attn_stk.close()

        # ---------- normalize attn^T by 1/Z (K=1 broadcast matmuls) ----------
        # dst base partition must be 0, so broadcast each head-group row into
        # [32, g, 512] psum and multiply with a partition-shifted DVE op.
        z_ps = ep(tc.tile_pool(name="z_ps", bufs=1, space="PSUM"))
        for dt in range(2):
            for ch in range(2):
                zx = z_ps.tile([32, 4, 512], F32, tag="zx")
                for g in range(4):
                    nc.tensor.matmul(
                        zx[0:32, g, :],
                        _r(ones1[0:1, 0:32]),
                        _r(zinv[0:1, 4 * dt + g, 512 * ch:512 * ch + 512]),
                        start=True, stop=True, tile_position=(0, 0))
                for g in range(4):
                    sl = (slice(32 * g, 32 * g + 32), dt,
                          slice(512 * ch, 512 * ch + 512))
                    nc.vector.tensor_mul(oT_sb[sl], oT_sb[sl], zx[0:32, g, :])

        # ---------- output: Wo @ attn^T + Wofc @ x^T + bfo + x ----------
        fin_ps = ep(tc.tile_pool(name="fin_ps", bufs=2, space="PSUM"))
        for ct in range(2):
            for ch in range(2):
                ps = fin_ps.tile([128, 512], F32, tag="fin")
                first = True
                for (w_sb, rhs_sb) in ((wo_sb, oT_sb), (wofc_sb, x_sb)):
                    for kt in range(2):
                        nc.tensor.matmul(
                            ps[:],
                            _r(w_sb[:, kt, 128 * ct:128 * ct + 128]),
                            _r(rhs_sb[:, kt, 512 * ch:512 * ch + 512]),
                            start=first, stop=(w_sb is wofc_sb and kt == 1))
                        first = False
                sl = (slice(None), ct, slice(512 * ch, 512 * ch + 512))
                nc.vector.tensor_add(out_sb[sl], ps[:],
                                     x_sb[:, ct, 512 * ch:512 * ch + 512])
                nc.vector.tensor_scalar_add(out_sb[sl], out_sb[sl],
                                            bpp[:, 4 + ct:4 + ct + 1])
        nc.sync.dma_start(out_d[:].rearrange("(k p) n -> p k n", p=128),
                          out_sb[:])


_CACHE = {}


def _get_nc(loop_input=False, variant="full"):
    key = (loop_input, variant)
    if key not in _CACHE:
        _CACHE[key] = build(loop_input, variant)
    return _CACHE[key]


def make_in_maps(x, pos_code, Wq, bq, Wk, bk, Wv, bv, Wo, bo, Wc, Wf, bf,
                 extra=None):
    x = np.asarray(x, np.float32)
    pos_code = np.asarray(pos_code, np.float32)
    wqcT = np.ascontiguousarray((np.asarray(Wq) @ np.asarray(Wc)).T, np.float32)
    wkT = np.ascontiguousarray(np.asarray(Wk).T, np.float32)
    # augmented V: per head 32 value cols + a zero col (ones come from bias)
    wvT = np.zeros((D, H * (DK + 1)), np.float32)
    brow = np.zeros((1, H * (DK + 1)), np.float32)
    vT = np.asarray(Wv).T
    bv_np = np.asarray(bv, np.float32)
    for h in range(H):
        wvT[:, (DK + 1) * h:(DK + 1) * h + DK] = vT[:, DK * h:DK * h + DK]
        brow[0, (DK + 1) * h:(DK + 1) * h + DK] = bv_np[DK * h:DK * h + DK]
        brow[0, (DK + 1) * h + DK] = 1.0
    wofcT = np.ascontiguousarray(
        (np.asarray(Wo) @ np.asarray(Wf) @ np.asarray(Wc)).T, np.float32)
    woT = np.ascontiguousarray(np.asarray(Wo).T, np.float32)
    bfo = (np.asarray(Wo) @ np.asarray(bf) + np.asarray(bo)).astype(np.float32)
    b_pp = np.stack([np.asarray(bq, np.float32).reshape(2, 128)[0],
                     np.asarray(bq, np.float32).reshape(2, 128)[1],
                     np.asarray(bk, np.float32).reshape(2, 128)[0],
                     np.asarray(bk, np.float32).reshape(2, 128)[1],
                     bfo.reshape(2, 128)[0],
                     bfo.reshape(2, 128)[1]], axis=1)
    b_pp = np.ascontiguousarray(b_pp, np.float32)          # [128, 6]

    B = x.shape[0]
    in_maps = []
    for b in range(B):
        m = {
            "x": np.ascontiguousarray(x[b].reshape(D, N)),
            "pos": np.ascontiguousarray(pos_code[b].reshape(D, N)),
            "wqcT": wqcT, "wkT": wkT, "wvT": wvT, "wofcT": wofcT,
            "woT": woT, "b_pp": b_pp, "b_row": brow,
            "ones1": np.ones((1, 128), np.float32),
        }
        if extra:
            m.update(extra)
        in_maps.append(m)
    return in_maps


def kernel(**inputs):
    nc = _get_nc(False, "batched")
    in_maps = make_in_maps(**inputs)
    res = run_bass_kernel_spmd(nc, in_maps, core_ids=list(range(NCORES)),
                               trace=False)
    out = np.stack([r["out"].reshape(D, N).T for r in res.results], axis=0)
    return np.ascontiguousarray(out, np.float32)



# revision 23
# speedup vs baseline: 1.0740x; 1.0740x over previous
"""Trainium2 Bass kernel for AttentionalPositionEncoding.

Reference computation (per batch b, with x_tok = x.reshape(C, N).T):
    cnn   = x_tok @ Wc.T
    q     = cnn @ Wq.T + bq           -> heads [h=8, N=1024, dk=32]
    k     = pos @ Wk.T + bk
    v     = pos @ Wv.T + bv
    attn  = softmax(q k^T / sqrt(dk)) @ v
    out   = (cnn @ Wf.T + bf + attn) @ Wo.T + bo + x_tok

Sharding: data-parallel over B=8 across the 8 NeuronCores (1 batch/core).

Host-side weight folding (exact algebra, done in fp32):
    Wqc  = Wq @ Wc          (q     = x_tok @ Wqc.T + bq)
    Wofc = Wo @ Wf @ Wc     (ffn   = x_tok @ Wofc.T)
    bfo  = Wo @ bf + bo

On-chip layout is feature-major ("CN": features on partitions, tokens on
free dim), which matches the HBM layout of x/pos ([C, H*W]) so no input
transposes are needed.  Attention scores are computed *transposed*
(S^T[j, i], keys on psum partitions) so that exp(S^T) feeds the P@V
matmul directly as the moving operand.  Softmax is unnormalized
(scores are O(8), exp is safe in fp32); the row sums Z are produced by a
ones-column appended to V (M=33 stationary), and 1/Z is applied after
P@V via a small select-matrix broadcast matmul.

The default execution path is the "batched" variant: per head pair, all
32 score matmuls + 16 [128,1024] exps run first (single PE tile mode,
ACT-paced), then all 32 P@V matmuls run back-to-back with contiguous
PSUM accumulation chains.  All matmuls are fp32r (full-rate fp32),
measured end-to-end relative error vs the fp32 reference: 1.6e-4.
Local measurement: ~190 us/kernel on one NeuronCore (8 cores run
data-parallel, one batch each).
"""

import math

import numpy as np

import concourse.bacc as bacc
import concourse.mybir as mybir
import concourse.tile as tile
from concourse.bass_utils import run_bass_kernel_spmd

F32 = mybir.dt.float32
F32R = mybir.dt.float32r
BF16 = mybir.dt.bfloat16

D = 256          # d_model
H = 8            # heads
DK = 32          # head dim
N = 1024         # tokens (32*32)
NCORES = 8
SCALE = 1.0 / math.sqrt(DK)


def _r(ap):
    """Bitcast an AP to float32r so the PE runs at 1 cycle/row."""
    return ap.bitcast(F32R)


def build(loop_input=False, variant="full"):
    """Build the per-core Bass program.

    loop_input=True adds a uint32 [1,1] input "niter" and wraps the whole
    body in a dynamic For_i — used by the local timing harness only.
    variant: "full" | ablations used for local perf attribution.
    """
    nc = bacc.Bacc(None, target_bir_lowering=False)

    x_d = nc.dram_tensor("x", [D, N], F32R, kind="ExternalInput")
    pos_d = nc.dram_tensor("pos", [D, N], F32R, kind="ExternalInput")
    wqcT_d = nc.dram_tensor("wqcT", [D, D], F32R, kind="ExternalInput")
    wkT_d = nc.dram_tensor("wkT", [D, D], F32R, kind="ExternalInput")
    # v weights augmented with a zero 33rd column per head; the ones come
    # from the bias row, so P@V also produces the softmax denominators Z.
    wvT_d = nc.dram_tensor("wvT", [D, H * (DK + 1)], F32R, kind="ExternalInput")
    wofcT_d = nc.dram_tensor("wofcT", [D, D], F32R, kind="ExternalInput")
    woT_d = nc.dram_tensor("woT", [D, D], F32R, kind="ExternalInput")
    bpp_d = nc.dram_tensor("b_pp", [128, 6], F32, kind="ExternalInput")
    brow_d = nc.dram_tensor("b_row", [1, H * (DK + 1)], F32R,
                            kind="ExternalInput")
    ones_d = nc.dram_tensor("ones1", [1, 128], F32R, kind="ExternalInput")
    out_d = nc.dram_tensor("out", [D, N], F32, kind="ExternalOutput")
    if loop_input:
        niter_d = nc.dram_tensor("niter", [1, 1], mybir.dt.uint32,
                                 kind="ExternalInput")

    with tile.TileContext(nc) as tc:
        import contextlib
        with contextlib.ExitStack() as stk:
            if loop_input:
                cpool = stk.enter_context(tc.tile_pool(name="cfg", bufs=1))
                nit_sb = cpool.tile([1, 1], mybir.dt.uint32)
                nc.sync.dma_start(nit_sb[:], niter_d[:])
                nit = nc.values_load(nit_sb[0:1, 0:1], min_val=1,
                                     max_val=1 << 20,
                                     skip_runtime_bounds_check=True)
                loop_cm = tc.For_i(0, nit, 1,
                                   hint_engines=tuple(mybir.ALL_ENGINES))
            else:
                loop_cm = contextlib.nullcontext()
            with loop_cm:
                _body(nc, tc, x_d, pos_d, wqcT_d, wkT_d, wvT_d, wofcT_d,
                      woT_d, bpp_d, brow_d, ones_d, out_d, variant)
    nc.compile()
    return nc


def _body(nc, tc, x_d, pos_d, wqcT_d, wkT_d, wvT_d, wofcT_d, woT_d,
          bpp_d, brow_d, ones_d, out_d, variant="full"):
    import contextlib
    with contextlib.ExitStack() as stk:
        ep = stk.enter_context

        persist = ep(tc.tile_pool(name="persist", bufs=1))

        # ---------- load inputs ----------
        def load_cn(dram, name):
            t = persist.tile([128, 2, dram.shape[1]], F32R, tag=name)
            nc.sync.dma_start(t[:], dram[:].rearrange("(k p) n -> p k n", p=128))
            return t

        x_sb = load_cn(x_d, "x_sb")          # [128, kt, 1024]
        pos_sb = load_cn(pos_d, "pos_sb")
        wqc_sb = load_cn(wqcT_d, "wqc_sb")   # [128, kt, 256]
        wk_sb = load_cn(wkT_d, "wk_sb")
        wv_sb = load_cn(wvT_d, "wv_sb")
        wofc_sb = load_cn(wofcT_d, "wofc_sb")
        wo_sb = load_cn(woT_d, "wo_sb")
        bpp = persist.tile([128, 6], F32, tag="bpp")
        nc.sync.dma_start(bpp[:], bpp_d[:])
        brow = persist.tile([1, H * (DK + 1)], F32R, tag="brow")
        nc.sync.dma_start(brow[:], brow_d[:])

        # constants
        ones1 = persist.tile([1, 128], F32R, tag="ones1")
        nc.sync.dma_start(ones1[:], ones_d[:])
        zbias = persist.tile([128, 1], F32, tag="zbias")
        nc.gpsimd.memset(zbias[:], 0.0)

        # persistent activations
        v2 = variant.startswith("v2") or variant == "pipelined"
        pv_dt = BF16 if ("bf16pv" in variant or v2) else F32R
        qk_dt = BF16 if variant.startswith("v2") else F32R
        q_sb = persist.tile([128, 2, N], qk_dt, tag="q_sb")
        k_sb = persist.tile([128, 2, N], qk_dt, tag="k_sb")
        v_aug = persist.tile([128, 8, H * (DK + 1)], pv_dt, tag="v_aug")
        oT_sb = persist.tile([128, 2, N], F32R, tag="oT_sb")
        # 1/Z rows, all on partition 0 (engine APs need 32-aligned bases)
        zinv = persist.tile([1, H, N], F32R, tag="zinv")
        out_sb = persist.tile([128, 2, N], F32, tag="out_sb")

        # ---------- q / k projections (CN layout) ----------
        with tc.tile_pool(name="dense_ps", bufs=2, space="PSUM") as dense_ps:
            for (dst, w_sb, rhs_sb, bcol) in ((q_sb, wqc_sb, x_sb, 0),
                                              (k_sb, wk_sb, pos_sb, 2)):
                for mt in range(2):
                    for ch in range(2):
                        ps = dense_ps.tile([128, 512], F32, tag="dense")
                        for kt in range(2):
                            nc.tensor.matmul(
                                ps[:],
                                _r(w_sb[:, kt, 128 * mt:128 * mt + 128]),
                                _r(rhs_sb[:, kt, 512 * ch:512 * ch + 512]),
                                start=(kt == 0), stop=(kt == 1))
                        with nc.allow_low_precision(reason="qk dtype knob"):
                            nc.vector.tensor_scalar_add(
                                dst[:, mt, 512 * ch:512 * ch + 512], ps[:],
                                bpp[:, bcol + mt:bcol + mt + 1])

            # ------- v projection (token-major, ones-augmented) -------
            for jt in range(8):
                ps = dense_ps.tile([128, H * (DK + 1)], F32, tag="dense")
                for kt in range(2):
                    nc.tensor.matmul(
                        ps[:],
                        _r(pos_sb[:, kt, 128 * jt:128 * jt + 128]),
                        _r(wv_sb[:, kt, :]),
                        start=(kt == 0), stop=False)
                nc.tensor.matmul(ps[:], _r(ones1[:]), _r(brow[:]),
                                 start=False, stop=True)
                with nc.allow_low_precision(reason="pv dtype knob"):
                    nc.vector.tensor_copy(v_aug[:, jt, :], ps[:])

        # ---------- attention: head pairs ----------
        attn_stk = stk.enter_context(contextlib.ExitStack())
        sc_ps = attn_stk.enter_context(
            tc.tile_pool(name="sc_ps", bufs=2, space="PSUM"))
        pv_ps = attn_stk.enter_context(
            tc.tile_pool(name="pv_ps", bufs=1, space="PSUM"))
        n_et = 34 if variant == "pipelined" else (
            18 if (variant.startswith("batched")
                   or variant.startswith("v2")) else 3)
        e_pool = attn_stk.enter_context(tc.tile_pool(name="e_pool", bufs=n_et))

        if variant.startswith("v2") and variant != "v2noattn":
            # bf16 attention: per-head score tiles with N=1024 streams.
            for hp in range(4):
                hA, hB = 2 * hp, 2 * hp + 1
                dt = hA // 4
                pA, pB = 32 * (hA % 4), 32 * (hB % 4)
                pvA = pv_ps.tile([128, N], F32, tag="pvA")
                pvB = pv_ps.tile([128, N], F32, tag="pvB")
                ets = {}
                # phase 1: scores + exp, one PE mode (32-row tiles)
                for jt in range(8):
                    for (h0, p0) in ((hA, pA), (hB, pB)):
                        sc = sc_ps.tile([128, 1024], F32, tag="sc")
                        nc.tensor.matmul(
                            sc[:],
                            k_sb[p0:p0 + 32, dt, 128 * jt:128 * jt + 128],
                            q_sb[p0:p0 + 32, dt, :],
                            start=True, stop=True, tile_position=(p0, 0))
                        et = e_pool.tile([128, 1024], BF16, tag="et")
                        with nc.allow_low_precision(reason="bf16 attention"):
                            nc.scalar.activation(
                                et[:], sc[:],
                                mybir.ActivationFunctionType.Exp,
                                bias=zbias[:, 0:1], scale=SCALE)
                        ets[(h0, jt)] = et
                # phase 2: P@V, one PE mode, contiguous accum chains
                for (h0, pvt) in ((hA, pvA), (hB, pvB)):
                    for jt in range(8):
                        nc.tensor.matmul(
                            pvt[0:DK + 1, :],
                            v_aug[:, jt,
                                  (DK + 1) * h0:(DK + 1) * h0 + DK + 1],
                            ets[(h0, jt)][:],
                            start=(jt == 0), stop=(jt == 7),
                            tile_position=(0, 0))
                nc.vector.tensor_copy(oT_sb[pA:pA + 32, dt, :], pvA[0:32, :])
                nc.vector.tensor_copy(oT_sb[pB:pB + 32, dt, :], pvB[0:32, :])
                with nc.allow_low_precision(reason="f32r full 32-bit width"):
                    nc.vector.reciprocal(zinv[0:1, hA, :], pvA[32:33, :])
                    nc.vector.reciprocal(zinv[0:1, hB, :], pvB[32:33, :])

        if variant == "pipelined":
            # Software-pipelined pairs: pair p's P@V matmuls are emitted
            # after pair p+1's scores+exp, so the PE fills the ACT-paced
            # stretches with P@V work instead of idling.  E and V in bf16
            # so two pairs of exp outputs fit in SBUF.
            def phase1(hp):
                hA, hB = 2 * hp, 2 * hp + 1
                dt = hA // 4
                pA, pB = 32 * (hA % 4), 32 * (hB % 4)
                ets = []
                for jt in range(8):
                    for ch in range(2):
                        sc = sc_ps.tile([128, 1024], F32, tag="sc")
                        for (h0, p0, lo) in ((hA, pA, 0), (hB, pB, 512)):
                            nc.tensor.matmul(
                                sc[:, lo:lo + 512],
                                _r(k_sb[p0:p0 + 32, dt,
                                        128 * jt:128 * jt + 128]),
                                _r(q_sb[p0:p0 + 32, dt,
                                        512 * ch:512 * ch + 512]),
                                start=True, stop=True, tile_position=(p0, 0))
                        et = e_pool.tile([128, 1024], BF16, tag="et")
                        with nc.allow_low_precision(reason="bf16 E"):
                            nc.scalar.activation(
                                et[:], sc[:],
                                mybir.ActivationFunctionType.Exp,
                                bias=zbias[:, 0:1], scale=SCALE)
                        ets.append(et)
                return ets

            def phase2(hp, ets):
                hA, hB = 2 * hp, 2 * hp + 1
                dt = hA // 4
                pA, pB = 32 * (hA % 4), 32 * (hB % 4)
                pvA = pv_ps.tile([128, N], F32, tag="pvA")
                pvB = pv_ps.tile([128, N], F32, tag="pvB")
                for (h0, elo, pvt) in ((hA, 0, pvA), (hB, 512, pvB)):
                    for ch in range(2):
                        for jt in range(8):
                            nc.tensor.matmul(
                                pvt[0:DK + 1, 512 * ch:512 * ch + 512],
                                v_aug[:, jt,
                                      (DK + 1) * h0:(DK + 1) * h0 + DK + 1],
                                ets[jt * 2 + ch][:, elo:elo + 512],
                                start=(jt == 0), stop=(jt == 7),
                                tile_position=(0, 0))
                nc.vector.tensor_copy(oT_sb[pA:pA + 32, dt, :], pvA[0:32, :])
                nc.vector.tensor_copy(oT_sb[pB:pB + 32, dt, :], pvB[0:32, :])
                with nc.allow_low_precision(reason="f32r full width"):
                    nc.vector.reciprocal(zinv[0:1, hA, :], pvA[32:33, :])
                    nc.vector.reciprocal(zinv[0:1, hB, :], pvB[32:33, :])

            prev = None
            for hp in range(4):
                ets = phase1(hp)
                if prev is not None:
                    phase2(prev[0], prev[1])
                prev = (hp, ets)
            phase2(prev[0], prev[1])

        if variant.startswith("batched"):
            for hp in range(4):
                hA, hB = 2 * hp, 2 * hp + 1
                dt = hA // 4
                pA, pB = 32 * (hA % 4), 32 * (hB % 4)
                pvA = pv_ps.tile([128, N], F32, tag="pvA")
                pvB = pv_ps.tile([128, N], F32, tag="pvB")
                ets = []
                # phase 1: all scores + exp for the pair (one PE mode)
                for jt in range(8):
                    for ch in range(2):
                        sc = sc_ps.tile([128, 1024], F32, tag="sc")
                        for (h0, p0, lo) in ((hA, pA, 0), (hB, pB, 512)):
                            nc.tensor.matmul(
                                sc[:, lo:lo + 512],
                                _r(k_sb[p0:p0 + 32, dt,
                                        128 * jt:128 * jt + 128]),
                                _r(q_sb[p0:p0 + 32, dt,
                                        512 * ch:512 * ch + 512]),
                                start=True, stop=True, tile_position=(p0, 0))
                        et = e_pool.tile([128, 1024], pv_dt, tag="et")
                        with nc.allow_low_precision(reason="pv dtype knob"):
                            nc.scalar.activation(
                                et[:], sc[:],
                                mybir.ActivationFunctionType.Exp,
                                bias=zbias[:, 0:1], scale=SCALE)
                        ets.append(et)
                # phase 2: all P@V for the pair (one PE mode, each psum
                # region's accumulation chain contiguous: BEGIN..MID..END)
                for (h0, elo, pvt) in ((hA, 0, pvA), (hB, 512, pvB)):
                    for ch in range(2):
                        for jt in range(8):
                            et = ets[jt * 2 + ch]
                            grp = ((jt == 0), (jt == 7))
                            if variant == "batched_nogroup":
                                grp = (True, True)
                            nc.tensor.matmul(
                                pvt[0:DK + 1, 512 * ch:512 * ch + 512],
                                v_aug[:, jt,
                                      (DK + 1) * h0:(DK + 1) * h0 + DK + 1],
                                et[:, elo:elo + 512],
                                start=grp[0], stop=grp[1],
                                tile_position=(0, 0))
                nc.vector.tensor_copy(oT_sb[pA:pA + 32, dt, :], pvA[0:32, :])
                nc.vector.tensor_copy(oT_sb[pB:pB + 32, dt, :], pvB[0:32, :])
                with nc.allow_low_precision(reason="f32r full 32-bit width"):
                    nc.vector.reciprocal(zinv[0:1, hA, :], pvA[32:33, :])
                    nc.vector.reciprocal(zinv[0:1, hB, :], pvB[32:33, :])

        for hp in (() if (variant.startswith("batched") or v2) else range(4)):
            hA, hB = 2 * hp, 2 * hp + 1
            dt = hA // 4
            pA, pB = 32 * (hA % 4), 32 * (hB % 4)
            # fp32r matmul dst base partition must be 0 (or 64 with M<=32):
            # give each head its own psum tile, both written at (0, 0), M=33.
            pvA = pv_ps.tile([128, N], F32, tag="pvA")
            pvB = pv_ps.tile([128, N], F32, tag="pvB")
            for jt in range(8):
                for ch in range(2):
                    sc = sc_ps.tile([128, 1024], F32, tag="sc")
                    # S^T tiles for heads A (cols 0:512) and B (cols 512:1024)
                    for (h0, p0, lo) in ((hA, pA, 0), (hB, pB, 512)):
                        nc.tensor.matmul(
                            sc[:, lo:lo + 512],
                            _r(k_sb[p0:p0 + 32, dt, 128 * jt:128 * jt + 128]),
                            _r(q_sb[p0:p0 + 32, dt, 512 * ch:512 * ch + 512]),
                            start=True, stop=True, tile_position=(p0, 0))
                    et = e_pool.tile([128, 1024], F32R, tag="et")
                    if variant == "expdve":
                        with nc.allow_low_precision(reason="perf ablation"):
                            nc.vector.tensor_copy(et[:], sc[:])
                    elif variant == "exp_sbuf":
                        st = e_pool.tile([128, 1024], F32, tag="st")
                        with nc.allow_low_precision(reason="perf ablation"):
                            nc.vector.tensor_copy(st[:], sc[:])
                        nc.scalar.activation(et[:], st[:],
                                             mybir.ActivationFunctionType.Exp,
                                             bias=zbias[:, 0:1], scale=SCALE)
                    else:
                        nc.scalar.activation(et[:], sc[:],
                                             mybir.ActivationFunctionType.Exp,
                                             bias=zbias[:, 0:1], scale=SCALE)
                    # P@V (+Z row at 32): accumulate over jt
                    if variant == "sconly":
                        if jt == 0:
                            for pvt in (pvA, pvB):
                                nc.tensor.matmul(
                                    pvt[0:DK + 1, 512 * ch:512 * ch + 512],
                                    _r(v_aug[:, jt, 0:DK + 1]),
                                    _r(et[:, 0:512]),
                                    start=True, stop=True,
                                    tile_position=(0, 0))
                    else:
                        for (h0, elo, pvt) in ((hA, 0, pvA), (hB, 512, pvB)):
                            grp = ((jt == 0), (jt == 7))
                            if variant == "batched_nogroup":
                                grp = (True, True)
                            nc.tensor.matmul(
                                pvt[0:DK + 1, 512 * ch:512 * ch + 512],
                                v_aug[:, jt,
                                      (DK + 1) * h0:(DK + 1) * h0 + DK + 1],
                                et[:, elo:elo + 512],
                                start=grp[0], stop=grp[1],
                                tile_position=(0, 0))
            # evacuate: unnormalized attn^T (CN) + 1/Z rows
            nc.vector.tensor_copy(oT_sb[pA:pA + 32, dt, :], pvA[0:32, :])
            nc.vector.tensor_copy(oT_sb[pB:pB + 32, dt, :], pvB[0:32, :])
            with nc.allow_low_precision(reason="f32r is full 32-bit width"):
                nc.vector.reciprocal(zinv[0:1, hA, :], pvA[32:33, :])
                nc.vector.reciprocal(zinv[0:1, hB, :], pvB[32:33, :])

        attn_stk.close()

        # ---------- normalize attn^T by 1/Z (K=1 broadcast matmuls) ----------
        # dst base partition must be 0, so broadcast each head-group row into
        # [32, g, 512] psum and multiply with a partition-shifted DVE op.
        z_ps = ep(tc.tile_pool(name="z_ps", bufs=1, space="PSUM"))
        for dt in range(2):
            for ch in range(2):
                zx = z_ps.tile([32, 4, 512], F32, tag="zx")
                for g in range(4):
                    nc.tensor.matmul(
                        zx[0:32, g, :],
                        _r(ones1[0:1, 0:32]),
                        _r(zinv[0:1, 4 * dt + g, 512 * ch:512 * ch + 512]),
                        start=True, stop=True, tile_position=(0, 0))
                for g in range(4):
                    sl = (slice(32 * g, 32 * g + 32), dt,
                          slice(512 * ch, 512 * ch + 512))
                    nc.vector.tensor_mul(oT_sb[sl], oT_sb[sl], zx[0:32, g, :])

        # ---------- output: Wo @ attn^T + Wofc @ x^T + bfo + x ----------
        fin_ps = ep(tc.tile_pool(name="fin_ps", bufs=2, space="PSUM"))
        for ct in range(2):
            for ch in range(2):
                ps = fin_ps.tile([128, 512], F32, tag="fin")
                first = True
                for (w_sb, rhs_sb) in ((wo_sb, oT_sb), (wofc_sb, x_sb)):
                    for kt in range(2):
                        nc.tensor.matmul(
                            ps[:],
                            _r(w_sb[:, kt, 128 * ct:128 * ct + 128]),
                            _r(rhs_sb[:, kt, 512 * ch:512 * ch + 512]),
                            start=first, stop=(w_sb is wofc_sb and kt == 1))
                        first = False
                sl = (slice(None), ct, slice(512 * ch, 512 * ch + 512))
                nc.vector.tensor_add(out_sb[sl], ps[:],
                                     x_sb[:, ct, 512 * ch:512 * ch + 512])
                nc.vector.tensor_scalar_add(out_sb[sl], out_sb[sl],
                                            bpp[:, 4 + ct:4 + ct + 1])
        nc.sync.dma_start(out_d[:].rearrange("(k p) n -> p k n", p=128),
                          out_sb[:])


# ====================================================================
# v3: single-stream pipelined kernel.
#
# Differences vs "batched":
#  * One 64-step attention pipeline (pair, jt, ch).  Each step: 2 score
#    matmuls (row-group packed at pA/pB), one exp over [128,1024], 2 PV
#    matmuls (col-group packed at 0/64, M=33 ones-augmented).  PV lags
#    scores by one step so the PE never waits on the exp engines.
#  * exp is split across ACT (true exp, bf16 out) and DVE (Schraudolph
#    bit-trick: int16(s*A + B) are the bf16 bits of ~exp(s*SCALE); the
#    softmax normalization cancels the sawtooth error; measured 9e-4
#    end-to-end in fp64 simulation).  Ratio tuned via DVE_EVERY.
#  * bf16 inputs/weights (x, pos, Wqc, Wk, Wv, Wofc, Wo), residual
#    folded host-side into Wofc+I, biases folded into ACT affine
#    (q/k), the ones-row matmul (v), and a K=1 ones matmul (final).
#  * Normalization: 1/Z per head (DVE reciprocal), broadcast via K=1
#    matmuls into the *unused* partitions of the same PV psum tile,
#    then one [128,1024] psum->sbuf bf16 copy + two bf16 2x-mode muls.
# ====================================================================

EXP_A = SCALE * math.log2(math.e) * 128.0      # folded softmax scale
EXP_B = 127.0 * 128.0 + 0.5                    # bf16 exponent bias + round
DVE_EVERY = 3                                  # every 3rd exp tile on DVE
I16 = mybir.dt.int16


def build_v3(loop_input=False, dve_every=DVE_EVERY, serial_pv=False):
    nc = bacc.Bacc(None, target_bir_lowering=False)

    x_d = nc.dram_tensor("x", [D, N], BF16, kind="ExternalInput")
    pos_d = nc.dram_tensor("pos", [D, N], BF16, kind="ExternalInput")
    wqcT_d = nc.dram_tensor("wqcT", [D, D], BF16, kind="ExternalInput")
    wkT_d = nc.dram_tensor("wkT", [D, D], BF16, kind="ExternalInput")
    wvT_d = nc.dram_tensor("wvT", [D, H * (DK + 1)], BF16,
                           kind="ExternalInput")
    wofcT_d = nc.dram_tensor("wofcT", [D, D], BF16, kind="ExternalInput")
    woT_d = nc.dram_tensor("woT", [D, D], BF16, kind="ExternalInput")
    bqk_d = nc.dram_tensor("bqk", [128, 4], F32, kind="ExternalInput")
    brow_d = nc.dram_tensor("b_row", [1, H * (DK + 1)], BF16,
                            kind="ExternalInput")
    bfo_d = nc.dram_tensor("bfo_pp", [128, 2], F32, kind="ExternalInput")
    ones16_d = nc.dram_tensor("ones16", [1, 512], BF16, kind="ExternalInput")
    out_d = nc.dram_tensor("out", [D, N], F32, kind="ExternalOutput")
    if loop_input:
        niter_d = nc.dram_tensor("niter", [1, 1], mybir.dt.uint32,
                                 kind="ExternalInput")

    with tile.TileContext(nc) as tc:
        import contextlib
        with contextlib.ExitStack() as stk:
            if loop_input:
                cpool = stk.enter_context(tc.tile_pool(name="cfg", bufs=1))
                nit_sb = cpool.tile([1, 1], mybir.dt.uint32)
                nc.sync.dma_start(nit_sb[:], niter_d[:])
                nit = nc.values_load(nit_sb[0:1, 0:1], min_val=1,
                                     max_val=1 << 20,
                                     skip_runtime_bounds_check=True)
                loop_cm = tc.For_i(0, nit, 1,
                                   hint_engines=tuple(mybir.ALL_ENGINES))
            else:
                loop_cm = contextlib.nullcontext()
            with loop_cm:
                _body_v3(nc, tc, x_d, pos_d, wqcT_d, wkT_d, wvT_d, wofcT_d,
                         woT_d, bqk_d, brow_d, bfo_d, ones16_d,
                         out_d, dve_every, serial_pv)
    nc.compile()
    return nc


def _body_v3(nc, tc, x_d, pos_d, wqcT_d, wkT_d, wvT_d, wofcT_d, woT_d,
             bqk_d, brow_d, bfo_d, ones16_d, out_d, dve_every,
             serial_pv=False):
    import contextlib
    Exp = mybir.ActivationFunctionType.Exp
    Ident = mybir.ActivationFunctionType.Identity
    with contextlib.ExitStack() as stk:
        ep = stk.enter_context

        persist = ep(tc.tile_pool(name="persist", bufs=1))

        def load_cn(dram, name, dt):
            t = persist.tile([128, 2, dram.shape[1]], dt, tag=name)
            nc.sync.dma_start(t[:], dram[:].rearrange("(k p) n -> p k n",
                                                      p=128))
            return t

        # load order: q-path first so dense work can start ASAP
        x_sb = load_cn(x_d, "x_sb", BF16)          # [128, kt, 1024]
        wqc_sb = load_cn(wqcT_d, "wqc_sb", BF16)   # [128, kt, 256]
        pos_sb = load_cn(pos_d, "pos_sb", BF16)
        wk_sb = load_cn(wkT_d, "wk_sb", BF16)
        wv_sb = load_cn(wvT_d, "wv_sb", BF16)      # [128, kt, 264]
        wofc_sb = load_cn(wofcT_d, "wofc_sb", BF16)
        wo_sb = load_cn(woT_d, "wo_sb", BF16)
        bqk = persist.tile([128, 4], F32, tag="bqk")
        nc.sync.dma_start(bqk[:], bqk_d[:])
        brow = persist.tile([1, H * (DK + 1)], BF16, tag="brow")
        nc.sync.dma_start(brow[:], brow_d[:])
        bfo_pp = persist.tile([128, 2], F32, tag="bfo_pp")
        nc.sync.dma_start(bfo_pp[:], bfo_d[:])
        ones16 = persist.tile([1, 512], BF16, tag="ones16")
        nc.sync.dma_start(ones16[:], ones16_d[:])

        q_sb = persist.tile([128, 2, N], F32R, tag="q_sb")
        k_sb = persist.tile([128, 2, N], F32R, tag="k_sb")
        v_aug = persist.tile([128, 8, H * (DK + 1)], BF16, tag="v_aug")
        oT_sb = persist.tile([128, 2, N], BF16, tag="oT_sb")
        out_sb = persist.tile([128, 2, N], F32, tag="out_sb")
        # zinv rows per head, all on partition 0 (32-aligned DVE shifts)
        zinv = persist.tile([1, H, N], BF16, tag="zinv")

        # ---------------- dense: q/k (ACT evac+bias), v ----------------
        with tc.tile_pool(name="dense_ps", bufs=3, space="PSUM") as dps:
            for mt in range(2):
                for (dst, w_sb, rhs_sb, bcol) in ((q_sb, wqc_sb, x_sb, 0),
                                                  (k_sb, wk_sb, pos_sb, 2)):
                    for ch in range(2):
                        ps = dps.tile([128, 512], F32, tag="qk")
                        for kt in range(2):
                            nc.tensor.matmul(
                                ps[:],
                                w_sb[:, kt, 128 * mt:128 * mt + 128],
                                rhs_sb[:, kt, 512 * ch:512 * ch + 512],
                                start=(kt == 0), stop=(kt == 1))
                        with nc.allow_low_precision(reason="f32r q/k"):
                            nc.scalar.activation(
                                dst[:, mt, 512 * ch:512 * ch + 512], ps[:],
                                Ident, bias=bqk[:, bcol + mt:bcol + mt + 1],
                                scale=1.0)
                if mt == 0:
                    for jt in range(8):
                        ps = dps.tile([128, H * (DK + 1)], F32, tag="v")
                        for kt in range(2):
                            nc.tensor.matmul(
                                ps[:],
                                pos_sb[:, kt, 128 * jt:128 * jt + 128],
                                wv_sb[:, kt, :],
                                start=(kt == 0), stop=False)
                        nc.tensor.matmul(ps[:], ones16[0:1, 0:128],
                                         brow[:], start=False, stop=True)
                        with nc.allow_low_precision(reason="bf16 V"):
                            nc.vector.tensor_copy(v_aug[:, jt, :], ps[:])

        # ---------------- attention pipeline ----------------
        attn = ep(contextlib.ExitStack())
        sc_ps = attn.enter_context(
            tc.tile_pool(name="sc_ps", bufs=2, space="PSUM"))
        pv_ps = attn.enter_context(
            tc.tile_pool(name="pv_ps", bufs=2 if serial_pv else 1,
                         space="PSUM"))
        e_pool = attn.enter_context(
            tc.tile_pool(name="e_pool", bufs=14 if serial_pv else 4))
        scr_pool = attn.enter_context(tc.tile_pool(name="scr", bufs=4))
        zx_pool = attn.enter_context(tc.tile_pool(name="zx", bufs=4))

        NS = 64  # steps: pair(4) x jt(8) x ch(2)

        def coords(s):
            hp, r = divmod(s, 16)
            if serial_pv:
                ch, jt = divmod(r, 8)   # ch-blocked production
            else:
                jt, ch = divmod(r, 2)
            hA, hB = 2 * hp, 2 * hp + 1
            return (hp, jt, ch, hA, hB, hp // 2,
                    32 * (hA % 4), 32 * (hB % 4))

        pvt = {}
        ets = {}
        scrt = {}

        def emit_score_exp(s):
            hp, jt, ch, hA, hB, dt, pA, pB = coords(s)
            if s % 16 == 0:
                # per-pair PV psum: A chains in banks 0-1 (rows 0-32),
                # B chains in banks 2-3 (rows 64-96).  One accumulation
                # group per bank (start=True clears the whole bank).
                # serial_pv: [128, 2, 512] — bank ch holds A's chain then
                # B's chain (rows 64-96) back-to-back, never concurrent.
                shape = [128, 2, 512] if serial_pv else [128, 4, 512]
                pvt[hp] = pv_ps.tile(shape, F32, tag="pv",
                                     name=f"pv{hp}")
            sc = sc_ps.tile([128, N], F32, tag="sc")
            for (p0, lo) in ((pA, 0), (pB, 512)):
                nc.tensor.matmul(
                    sc[:, lo:lo + 512],
                    k_sb[p0:p0 + 32, dt, 128 * jt:128 * jt + 128],
                    q_sb[p0:p0 + 32, dt, 512 * ch:512 * ch + 512],
                    start=True, stop=True, tile_position=(p0, 0))
            et = e_pool.tile([128, N], BF16, tag="et")
            if dve_every and (s % dve_every == dve_every - 1):
                with nc.allow_low_precision(reason="schraudolph exp bits"):
                    nc.vector.tensor_scalar(
                        et[:].bitcast(I16), sc[:], EXP_A, EXP_B,
                        op0=mybir.AluOpType.mult, op1=mybir.AluOpType.add)
            else:
                with nc.allow_low_precision(reason="bf16 attention weights"):
                    nc.scalar.activation(et[:], sc[:], Exp, bias=0.0,
                                         scale=SCALE)
            ets[s] = et

        def emit_pv(s):
            hp, jt, ch, hA, hB, dt, pA, pB = coords(s)
            et, pv = ets.pop(s), pvt[hp]
            for (h0, base, bank, lo, tp) in ((hA, 0, ch, 0, (0, 0)),
                                             (hB, 64, 2 + ch, 512, (0, 64))):
                nc.tensor.matmul(
                    pv[base:base + DK + 1, bank, :],
                    v_aug[:, jt, (DK + 1) * h0:(DK + 1) * h0 + DK + 1],
                    et[:, lo:lo + 512],
                    start=(jt == 0), stop=(jt == 7), tile_position=tp)

        def emit_evac1(hp):
            """Copy PV psum out (frees the pv tile) + 1/Z. DVE-only."""
            _, _, _, hA, hB, dt, pA, pB = coords(16 * hp)
            pv = pvt.pop(hp)
            scrA = scr_pool.tile([33, N], BF16, tag="scr", name=f"scrA{hp}")
            scrB = scr_pool.tile([33, N], BF16, tag="scr", name=f"scrB{hp}")
            bsl = pv[64:97, 0:2, :] if serial_pv else pv[64:97, 2:4, :]
            with nc.allow_low_precision(reason="bf16 attn out"):
                nc.vector.tensor_copy(scrA[0:33, :], pv[0:33, 0:2, :])
                nc.vector.tensor_copy(scrB[0:33, :], bsl)
            with nc.allow_low_precision(reason="zinv from bf16 Z"):
                nc.vector.reciprocal(zinv[0:1, hA, :], scrA[32:33, :])
                nc.vector.reciprocal(zinv[0:1, hB, :], scrB[32:33, :])
            scrt[hp] = (scrA, scrB)

        def emit_evac2(hp):
            """Broadcast 1/Z across partitions (gpsimd DMA) + normalize."""
            _, _, _, hA, hB, dt, pA, pB = coords(16 * hp)
            scrA, scrB = scrt.pop(hp)
            zxA = zx_pool.tile([32, N], BF16, tag="zx", name=f"zxA{hp}")
            zxB = zx_pool.tile([32, N], BF16, tag="zx", name=f"zxB{hp}")
            nc.gpsimd.partition_broadcast(zxA[0:32, :], zinv[0:1, hA, :])
            nc.gpsimd.partition_broadcast(zxB[0:32, :], zinv[0:1, hB, :])
            with nc.allow_low_precision(reason="bf16 attn out"):
                nc.vector.tensor_mul(oT_sb[pA:pA + 32, dt, :],
                                     scrA[0:32, :], zxA[0:32, :])
                nc.vector.tensor_mul(oT_sb[pB:pB + 32, dt, :],
                                     scrB[0:32, :], zxB[0:32, :])

        if serial_pv:
            # PV chains serialized per bank: A-ch then B-ch; 2 MMs/step
            # drained from a queue, gated on et availability.
            def emit_pv_entry(e):
                hp, head, ch, jt = e
                h0 = 2 * hp + head
                pv = pvt[hp]
                et = ets[16 * hp + 8 * ch + jt]
                base, lo = (0, 0) if head == 0 else (64, 512)
                tp = (0, 0) if head == 0 else (0, 64)
                nc.tensor.matmul(
                    pv[base:base + DK + 1, ch, :],
                    v_aug[:, jt, (DK + 1) * h0:(DK + 1) * h0 + DK + 1],
                    et[:, lo:lo + 512],
                    start=(jt == 0), stop=(jt == 7), tile_position=tp)

            pvq = []
            evac1_pend = []
            evac2_pend = []
            for s in range(NS + 10):
                if s < NS:
                    emit_score_exp(s)
                    if s % 16 == 0:
                        pvq.extend(
                            [(s // 16, head, ch, jt)
                             for ch in range(2) for head in range(2)
                             for jt in range(8)])
                if evac2_pend:
                    emit_evac2(evac2_pend.pop(0))
                drained = 0
                while (pvq and drained < 2
                       and (16 * pvq[0][0] + 8 * pvq[0][2] + pvq[0][3])
                       in ets):
                    e = pvq.pop(0)
                    emit_pv_entry(e)
                    drained += 1
                    if e[1] == 1 and e[2] == 1 and e[3] == 7:
                        evac1_pend.append(e[0])
                if evac1_pend:
                    hp_e = evac1_pend.pop(0)
                    emit_evac1(hp_e)
                    evac2_pend.append(hp_e)
            # ets keyed by s are never popped in serial mode; clear refs
            ets.clear()
        else:
            for s in range(NS + 1):
                if s < NS:
                    emit_score_exp(s)
                if s >= 1:
                    emit_pv(s - 1)
                if s % 16 == 0 and s >= 16:
                    emit_evac1(s // 16 - 1)
                if s % 16 == 2 and s >= 18:
                    emit_evac2(s // 16 - 1)
            emit_evac2(3)
        attn.close()

        # ---------------- final: Wo@oT + (Wofc+I)@x + bfo ----------------
        with tc.tile_pool(name="fin_ps", bufs=2, space="PSUM") as fin_ps:
            for ct in range(2):
                for ch in range(2):
                    ps = fin_ps.tile([128, 512], F32, tag="fin")
                    k = 0
                    for (w_sb, rhs_sb) in ((wo_sb, oT_sb), (wofc_sb, x_sb)):
                        for kt in range(2):
                            nc.tensor.matmul(
                                ps[:],
                                w_sb[:, kt, 128 * ct:128 * ct + 128],
                                rhs_sb[:, kt, 512 * ch:512 * ch + 512],
                                start=(k == 0), stop=(k == 3))
                            k += 1
                    nc.scalar.activation(
                        out_sb[:, ct, 512 * ch:512 * ch + 512], ps[:],
                        mybir.ActivationFunctionType.Identity,
                        bias=bfo_pp[:, ct:ct + 1], scale=1.0)
                nc.sync.dma_start(
                    out_d[:].rearrange("(k p) n -> p k n", p=128)[:, ct, :],
                    out_sb[:, ct, :])


def make_in_maps_v3(x, pos_code, Wq, bq, Wk, bk, Wv, bv, Wo, bo, Wc, Wf, bf,
                    extra=None):
    def b16(a):
        return np.asarray(a, dtype=np.float32).astype(
            mybir.dt.np(BF16))

    x = np.asarray(x, np.float32)
    pos_code = np.asarray(pos_code, np.float32)
    wqcT = b16((np.asarray(Wq) @ np.asarray(Wc)).T)
    wkT = b16(np.asarray(Wk).T)
    wvT = np.zeros((D, H * (DK + 1)), np.float32)
    brow = np.zeros((1, H * (DK + 1)), np.float32)
    vT = np.asarray(Wv).T
    bv_np = np.asarray(bv, np.float32)
    for h in range(H):
        wvT[:, (DK + 1) * h:(DK + 1) * h + DK] = vT[:, DK * h:DK * h + DK]
        brow[0, (DK + 1) * h:(DK + 1) * h + DK] = bv_np[DK * h:DK * h + DK]
        brow[0, (DK + 1) * h + DK] = 1.0
    wofcT = b16((np.asarray(Wo) @ np.asarray(Wf) @ np.asarray(Wc)
                 + np.eye(D, dtype=np.float32)).T)
    woT = b16(np.asarray(Wo).T)
    bfo = (np.asarray(Wo) @ np.asarray(bf) + np.asarray(bo)).astype(np.float32)
    bqk = np.stack([np.asarray(bq, np.float32).reshape(2, 128)[0],
                    np.asarray(bq, np.float32).reshape(2, 128)[1],
                    np.asarray(bk, np.float32).reshape(2, 128)[0],
                    np.asarray(bk, np.float32).reshape(2, 128)[1]], axis=1)
    bqk = np.ascontiguousarray(bqk, np.float32)            # [128, 4]

    B = x.shape[0]
    in_maps = []
    for b in range(B):
        m = {
            "x": b16(x[b].reshape(D, N)),
            "pos": b16(pos_code[b].reshape(D, N)),
            "wqcT": wqcT, "wkT": wkT, "wvT": b16(wvT), "wofcT": wofcT,
            "woT": woT, "bqk": bqk, "b_row": b16(brow),
            "bfo_pp": np.ascontiguousarray(bfo.reshape(2, 128).T),
            "ones16": b16(np.ones((1, 512))),
        }
        if extra:
            m.update(extra)
        in_maps.append(m)
    return in_maps


_CACHE = {}


def _get_nc(loop_input=False, variant="full"):
    key = (loop_input, variant)
    if key not in _CACHE:
        if variant.startswith("v3") or variant.startswith("v4"):
            dve_every = DVE_EVERY
            if "_d" in variant:
                dve_every = int(variant.split("_d")[1])
            _CACHE[key] = build_v3(loop_input, dve_every,
                                   serial_pv=variant.startswith("v4"))
        else:
            _CACHE[key] = build(loop_input, variant)
    return _CACHE[key]


def make_in_maps(x, pos_code, Wq, bq, Wk, bk, Wv, bv, Wo, bo, Wc, Wf, bf,
                 extra=None):
    x = np.asarray(x, np.float32)
    pos_code = np.asarray(pos_code, np.float32)
    wqcT = np.ascontiguousarray((np.asarray(Wq) @ np.asarray(Wc)).T, np.float32)
    wkT = np.ascontiguousarray(np.asarray(Wk).T, np.float32)
    # augmented V: per head 32 value cols + a zero col (ones come from bias)
    wvT = np.zeros((D, H * (DK + 1)), np.float32)
    brow = np.zeros((1, H * (DK + 1)), np.float32)
    vT = np.asarray(Wv).T
    bv_np = np.asarray(bv, np.float32)
    for h in range(H):
        wvT[:, (DK + 1) * h:(DK + 1) * h + DK] = vT[:, DK * h:DK * h + DK]
        brow[0, (DK + 1) * h:(DK + 1) * h + DK] = bv_np[DK * h:DK * h + DK]
        brow[0, (DK + 1) * h + DK] = 1.0
    wofcT = np.ascontiguousarray(
        (np.asarray(Wo) @ np.asarray(Wf) @ np.asarray(Wc)).T, np.float32)
    woT = np.ascontiguousarray(np.asarray(Wo).T, np.float32)
    bfo = (np.asarray(Wo) @ np.asarray(bf) + np.asarray(bo)).astype(np.float32)
    b_pp = np.stack([np.asarray(bq, np.float32).reshape(2, 128)[0],
                     np.asarray(bq, np.float32).reshape(2, 128)[1],
                     np.asarray(bk, np.float32).reshape(2, 128)[0],
                     np.asarray(bk, np.float32).reshape(2, 128)[1],
                     bfo.reshape(2, 128)[0],
                     bfo.reshape(2, 128)[1]], axis=1)
    b_pp = np.ascontiguousarray(b_pp, np.float32)          # [128, 6]

    B = x.shape[0]
    in_maps = []
    for b in range(B):
        m = {
            "x": np.ascontiguousarray(x[b].reshape(D, N)),
            "pos": np.ascontiguousarray(pos_code[b].reshape(D, N)),
            "wqcT": wqcT, "wkT": wkT, "wvT": wvT, "wofcT": wofcT,
            "woT": woT, "b_pp": b_pp, "b_row": brow,
            "ones1": np.ones((1, 128), np.float32),
        }
        if extra:
            m.update(extra)
        in_maps.append(m)
    return in_maps


KERNEL_VARIANT = "v3"


def kernel(**inputs):
    nc = _get_nc(False, KERNEL_VARIANT)
    if KERNEL_VARIANT.startswith("v3") or KERNEL_VARIANT.startswith("v4"):
        in_maps = make_in_maps_v3(**inputs)
    else:
        in_maps = make_in_maps(**inputs)
    res = run_bass_kernel_spmd(nc, in_maps, core_ids=list(range(NCORES)),
                               trace=False)
    out = np.stack([r["out"].reshape(D, N).T for r in res.results], axis=0)
    return np.ascontiguousarray(out, np.float32)

